# revision 21
# baseline (speedup 1.0000x reference)
"""Trainium2 Bass kernel for nn_Network_56873956933906 (gnn_message_passing).

Equivariant message-passing network (e3nn-style tensor-field network), 3 layers.
Data-parallel over the molecule batch: 32 molecules sharded 4-per-core across 8
NeuronCores. Each core computes its 4 molecules fully independently (no
collectives); the host gathers the 8 output shards.

Per-core algorithm ("G-route"):
  pair layout: partitions = (b,j) padded to (4 x 32); free dims carry (a, ...)
  1. geometry -> Y (16 real-SH comps), radial gaussians rb (10), all on-chip
  2. Filter[(b,j), (k,f,a)] = rb*Y outer product (one broadcast-AP vector op)
  3. stage 1 (PE):  G[vi, (k,f,a)] = fin^T @ Filter   per molecule row-group
  4. stage 2 (PE):  out[a, (u,o)] = sum_kf G_kf^T @ WQ_kf  (PSUM-accumulated)
     where WQ[vi,(k,f),(u,o)] = W[k,u,v]*Q[i,f,o] is built on-device from a
     host-side pure-permutation rearrangement of radial_w (wexp) times constant
     Clebsch-Gordan tables (qexp).
  5. gated nonlinearity -> next layer features.
"""

import sys
from math import factorial, sqrt

import numpy as np

for _p in ("/opt/trn_rl_repo", "/root/.axon_site/_ro/trn_rl_repo"):
    if _p not in sys.path:
        sys.path.append(_p)

# ----------------------------------------------------------------------------
# problem constants (hardcoded per spec)
# ----------------------------------------------------------------------------
B, N, EMBED, MAXZ = 32, 24, 32, 10
NB, RMAX = 10, 5.0
MULS = [24, 12, 8, 4]          # hidden multiplicities per l
U = [48, 12, 8, 4]             # gated output multiplicities per l
NF = 16                        # total SH components l=0..3
INV = 1.0 / sqrt(18.0)         # 1/sqrt(avg_n_atoms)
WIDTH = RMAX / (NB - 1)
VI_OFF = [0, 24, 60, 100, 128]     # hidden feature (v,i) block offsets per l
UO_OFF = [0, 48, 84, 124, 152]     # output (u,o) block offsets per l
F_OFF = [0, 1, 4, 9, 16]           # SH component offsets per l
NCORES = 8
BC = B // NCORES               # molecules per core
PAD = 32                       # padded atoms-per-molecule in partition layout
NP = BC * PAD                  # 128 partitions
L_LAYERS = 3

# ---------------------------------------------------------------------------
# path table + real Clebsch-Gordan tables (host constants; mirrors reference)
# ---------------------------------------------------------------------------
Rs = [[(EMBED, 0)]] + [[(m, l) for l, m in enumerate(MULS)]] * L_LAYERS


def _gated_rs(rs):
    n_gate = sum(m for m, l in rs if l > 0)
    return [(rs[0][0] + n_gate, 0)] + [(m, l) for m, l in rs if l > 0]


PATHS = []
_off = 0
for _layer in range(L_LAYERS):
    for _mi, _li in Rs[_layer]:
        for _mo, _lo in _gated_rs(Rs[_layer + 1]):
            for _lf in range(abs(_li - _lo), min(3, _li + _lo) + 1):
                PATHS.append((_layer, _li, _mi, _lo, _mo, _lf, _off))
                _off += NB * _mo * _mi
TOTAL_W = _off


def _cg(j1, m1, j2, m2, j3, m3):
    if m1 + m2 != m3:
        return 0.0
    f = factorial
    pre = sqrt((2 * j3 + 1) * f(j3 + j1 - j2) * f(j3 - j1 + j2) * f(j1 + j2 - j3)
               / f(j1 + j2 + j3 + 1))
    pre *= sqrt(f(j3 + m3) * f(j3 - m3) * f(j1 - m1) * f(j1 + m1) * f(j2 - m2) * f(j2 + m2))
    s = 0.0
    for k in range(j1 + j2 + j3 + 1):
        args = [k, j1 + j2 - j3 - k, j1 - m1 - k, j2 + m2 - k, j3 - j2 + m1 + k,
                j3 - j1 - m2 + k]
        if any(a < 0 for a in args):
            continue
        d = 1.0
        for a in args:
            d *= f(a)
        s += (-1) ** k / d
    return pre * s


def _u_real(l):
    Uc = np.zeros((2 * l + 1, 2 * l + 1), dtype=complex)
    s = 1.0 / np.sqrt(2.0)
    for m in range(-l, l + 1):
        if m == 0:
            Uc[l, l] = 1.0
        elif m > 0:
            Uc[m + l, m + l] = ((-1) ** m) * s
            Uc[m + l, -m + l] = s
        else:
            a = -m
            Uc[m + l, m + l] = 1j * s
            Uc[m + l, a + l] = -1j * ((-1) ** a) * s
    return Uc


def _real_cg(l1, l2, l3):
    Cc = np.zeros((2 * l1 + 1, 2 * l2 + 1, 2 * l3 + 1), dtype=complex)
    for m1 in range(-l1, l1 + 1):
        for m2 in range(-l2, l2 + 1):
            m3 = m1 + m2
            if -l3 <= m3 <= l3:
                Cc[m1 + l1, m2 + l2, m3 + l3] = _cg(l1, m1, l2, m2, l3, m3)
    T = np.einsum("ai,bj,ck,ijk->abc", _u_real(l1), _u_real(l2), _u_real(l3).conj(), Cc)
    Tr, Ti = np.real(T), np.imag(T)
    return np.asarray(Tr if np.abs(Tr).sum() >= np.abs(Ti).sum() else Ti, np.float32)


CG = {}
for _, _li, _, _lo, _, _lf, _ in PATHS:
    if (_li, _lf, _lo) not in CG:
        CG[(_li, _lf, _lo)] = _real_cg(_li, _lf, _lo)

_PATH_BY_KEY = {}
for p in PATHS:
    _PATH_BY_KEY[(p[0], p[1], p[3], p[5])] = p  # (layer, li, lo, lf) -> path

# (lf, lo) block orders + column offsets for wexp / qexp / WQ
LFLO_H = [(lf, lo) for lf in range(4) for lo in range(4)]
LFLO_0 = [(l, l) for l in range(4)]
WOFF_H, QOFF_H = {}, {}
_w = _q = 0
for (lf, lo) in LFLO_H:
    WOFF_H[(lf, lo)] = _w
    QOFF_H[(lf, lo)] = _q
    _w += NB * U[lo]
    _q += (2 * lf + 1) * (2 * lo + 1)
WEXP_H_COLS, QEXP_H_COLS = _w, _q          # 2880, 256

WOFF_0, QOFF_0, W0QOFF = {}, {}, {}
_w = _q = _c = 0
for (lf, lo) in LFLO_0:
    WOFF_0[(lf, lo)] = _w
    QOFF_0[(lf, lo)] = _q
    W0QOFF[(lf, lo)] = _c                   # compact WQ0 block offsets
    _w += NB * U[lo]
    _q += (2 * lf + 1) * (2 * lo + 1)
    _c += NB * U[lo] * (2 * lf + 1) * (2 * lo + 1)
WEXP_0_COLS, QEXP_0_COLS, WQ0_COLS = _w, _q, _c   # 720, 84, 5520

WQ_H_COLS = NB * NF * 152                   # 24320


# ---------------------------------------------------------------------------
# host-side constant tables / input marshalling
# ---------------------------------------------------------------------------
def _build_wexp(radial_w, layer):
    """Pure gather/permutation of radial_w (plus structural zeros)."""
    if layer == 0:
        out = np.zeros((32, WEXP_0_COLS), np.float32)
        for (lf, lo) in LFLO_0:
            pth = _PATH_BY_KEY.get((0, 0, lo, lf))
            if pth is None:
                continue
            _, li, mi, _, mo, _, off = pth
            W = radial_w[off:off + NB * mo * mi].reshape(NB, mo, mi)
            blk = W.transpose(2, 0, 1).reshape(mi, NB * mo)   # [v, (k,u)]
            out[:, WOFF_0[(lf, lo)]:WOFF_0[(lf, lo)] + NB * mo] = blk
        return out
    out = np.zeros((128, WEXP_H_COLS), np.float32)
    for (lf, lo) in LFLO_H:
        c0 = WOFF_H[(lf, lo)]
        for li in range(4):
            pth = _PATH_BY_KEY.get((layer, li, lo, lf))
            if pth is None:
                continue
            _, _, mi, _, mo, _, off = pth
            W = radial_w[off:off + NB * mo * mi].reshape(NB, mo, mi)
            blk = W.transpose(2, 0, 1).reshape(mi, NB * mo)   # [v, (k,u)]
            blk = np.repeat(blk, 2 * li + 1, axis=0)          # [(v,i), (k,u)]
            out[VI_OFF[li]:VI_OFF[li + 1], c0:c0 + NB * mo] = blk
    return out


def _build_qexp():
    """Constant CG tables with all normalization folded in."""
    q0 = np.zeros((32, QEXP_0_COLS), np.float32)
    for (lf, lo) in LFLO_0:
        Q = CG[(0, lf, lo)].reshape(1, -1)                    # [(i=1), (mf,o)]
        scale = INV / sqrt(NB * EMBED)
        q0[:, QOFF_0[(lf, lo)]:QOFF_0[(lf, lo)] + Q.shape[1]] = (
            np.tile(Q, (32, 1)) * scale)
    qh = np.zeros((128, QEXP_H_COLS), np.float32)
    for (lf, lo) in LFLO_H:
        c0 = QOFF_H[(lf, lo)]
        ncols = (2 * lf + 1) * (2 * lo + 1)
        for li in range(4):
            if (1, li, lo, lf) not in _PATH_BY_KEY:
                continue
            Q = CG[(li, lf, lo)].reshape(2 * li + 1, ncols)   # [i, (mf,o)]
            scale = INV / sqrt(NB * MULS[li])
            qh[VI_OFF[li]:VI_OFF[li + 1], c0:c0 + ncols] = (
                np.tile(Q, (MULS[li], 1)) * scale)
    return q0, qh


_Q0_CONST, _QH_CONST = _build_qexp()
_CGRID_CONST = np.tile(
    np.repeat(np.linspace(0.0, RMAX, NB).astype(np.float32), N)[None, :], (NP, 1))
_IOTA10_CONST = np.arange(MAXZ, dtype=np.float32)[:, None]

# dtype knobs
WQ_BF16_IN = True       # wexp/qexp shipped as bf16 (2x DVE build throughput)
STAGE1_F32R = True      # stage-1 matmul in fp32r (full-rate, near-fp32)


# ---------------------------------------------------------------------------
# device program
# ---------------------------------------------------------------------------
def _build_nc(debug_taps=()):
    import concourse.bacc as bacc
    import concourse.bass as bass
    import concourse.mybir as mybir
    from concourse.tile import TileContext

    dt = mybir.dt
    ALU = mybir.AluOpType
    ACT = mybir.ActivationFunctionType
    wq_dt = dt.bfloat16
    wq_in_dt = dt.bfloat16 if WQ_BF16_IN else dt.float32
    s1_dt = dt.float32r if STAGE1_F32R else dt.bfloat16

    nc = bacc.Bacc("TRN2", target_bir_lowering=False, debug=False)

    # ---- DRAM I/O ----
    d_zrep = nc.dram_tensor("zrep", [MAXZ, NP], dt.float32, kind="ExternalInput")
    d_posj = nc.dram_tensor("posj", [NP, 3], dt.float32, kind="ExternalInput")
    d_posa = nc.dram_tensor("posa", [NP, N * 3], dt.float32, kind="ExternalInput")
    d_mask = nc.dram_tensor("maskp", [NP, 1], dt.float32, kind="ExternalInput")
    d_embed = nc.dram_tensor("embed", [MAXZ, EMBED], dt.float32, kind="ExternalInput")
    d_cgrid = nc.dram_tensor("cgrid", [NP, NB * N], dt.float32, kind="ExternalInput")
    d_iota = nc.dram_tensor("iota10", [MAXZ, 1], dt.float32, kind="ExternalInput")
    d_cbias = nc.dram_tensor("cbias", [128, 2], dt.float32, kind="ExternalInput")
    d_wexp = [
        nc.dram_tensor("wexp0", [32, WEXP_0_COLS], wq_in_dt, kind="ExternalInput"),
        nc.dram_tensor("wexp1", [128, WEXP_H_COLS], wq_in_dt, kind="ExternalInput"),
        nc.dram_tensor("wexp2", [128, WEXP_H_COLS], wq_in_dt, kind="ExternalInput"),
    ]
    d_q0 = nc.dram_tensor("qexp0", [32, QEXP_0_COLS], wq_in_dt, kind="ExternalInput")
    d_qh = nc.dram_tensor("qexph", [128, QEXP_H_COLS], wq_in_dt, kind="ExternalInput")
    d_out = nc.dram_tensor("out", [BC * N, 128], dt.float32, kind="ExternalOutput")

    with TileContext(nc) as tc:
        with (
            tc.tile_pool(name="cpool", bufs=1) as cpool,
            tc.tile_pool(name="gpool", bufs=1) as gpool,
            tc.tile_pool(name="wqpool", bufs=1) as wqpool,
            tc.tile_pool(name="psA", bufs=5, space="PSUM") as psA,
            tc.tile_pool(name="psB", bufs=2, space="PSUM") as psB,
        ):
            def tap(name, ap):
                if name not in debug_taps:
                    return
                dtt = ap.dtype
                if dtt == dt.float32r:
                    ap = ap.bitcast(dt.float32)
                    dtt = dt.float32
                if ap.space == bass.MemorySpace.PSUM:
                    bounce = cpool.tile(list(ap.shape), dtt, name=f"tapb_{name}")
                    nc.vector.tensor_copy(bounce[:], ap)
                    ap = bounce[:]
                dtap = nc.dram_tensor(f"tap_{name}", list(ap.shape), dtt,
                                      kind="ExternalOutput")
                nc.sync.dma_start(dtap.ap(), ap)

            # ================= loads =================
            zrep = cpool.tile([MAXZ, NP], dt.float32)
            nc.sync.dma_start(zrep[:], d_zrep.ap())
            posj = cpool.tile([NP, 3], dt.float32)
            nc.sync.dma_start(posj[:], d_posj.ap())
            posa = cpool.tile([NP, N * 3], dt.float32)
            nc.sync.dma_start(posa[:], d_posa.ap())
            maskp = cpool.tile([NP, 1], dt.float32)
            nc.sync.dma_start(maskp[:], d_mask.ap())
            embed = cpool.tile([MAXZ, EMBED], dt.float32)
            nc.sync.dma_start(embed[:], d_embed.ap())
            cgrid = cpool.tile([NP, NB * N], dt.float32)
            nc.sync.dma_start(cgrid[:], d_cgrid.ap())
            iota10 = cpool.tile([MAXZ, 1], dt.float32)
            nc.sync.dma_start(iota10[:], d_iota.ap())
            cbias = cpool.tile([128, 2], dt.float32)
            nc.sync.dma_start(cbias[:], d_cbias.ap())
            nc.const_aps.aps[(dt.float32, 0.0)] = cbias[:, 0:1]
            nc.const_aps.aps[(dt.float32, 1e-12)] = cbias[:, 1:2]
            wexp = []
            for L in range(3):
                sh = [32, WEXP_0_COLS] if L == 0 else [128, WEXP_H_COLS]
                t = cpool.tile(sh, wq_in_dt, name=f"wexp{L}s")
                nc.sync.dma_start(t[:], d_wexp[L].ap())
                wexp.append(t)
            q0 = cpool.tile([32, QEXP_0_COLS], wq_in_dt)
            nc.sync.dma_start(q0[:], d_q0.ap())
            qh = cpool.tile([128, QEXP_H_COLS], wq_in_dt)
            nc.sync.dma_start(qh[:], d_qh.ap())

            # ================= pair geometry =================
            # diff[(b,j), (a,c)] = pos[b,j,c] - pos[b,a,c]
            diff = cpool.tile([NP, N * 3], dt.float32)
            nc.vector.tensor_tensor(
                diff[:].rearrange("p (a c) -> p a c", c=3),
                posj[:].unsqueeze(1).broadcast_to([NP, N, 3]),
                posa[:].rearrange("p (a c) -> p a c", c=3),
                op=ALU.subtract)
            sq = cpool.tile([NP, N * 3], dt.float32)
            nc.vector.tensor_mul(sq[:], diff[:], diff[:])
            r2 = cpool.tile([NP, N], dt.float32)
            nc.vector.reduce_sum(r2[:], sq[:].rearrange("p (a c) -> p a c", c=3),
                                 axis=mybir.AxisListType.X)
            radii = cpool.tile([NP, N], dt.float32)
            nc.scalar.activation(radii[:], r2[:], ACT.Sqrt, bias=1e-12, scale=1.0)
            radii9 = cpool.tile([NP, N], dt.float32)
            nc.vector.tensor_scalar_add(radii9[:], radii[:], 1e-9)
            rinv = cpool.tile([NP, N], dt.float32)
            nc.vector.reciprocal(rinv[:], radii9[:])
            unit = cpool.tile([NP, N * 3], dt.float32)
            nc.vector.tensor_tensor(
                unit[:].rearrange("p (a c) -> p a c", c=3),
                diff[:].rearrange("p (a c) -> p a c", c=3),
                rinv[:].unsqueeze(2).broadcast_to([NP, N, 3]),
                op=ALU.mult)

            # component views [NP, N] with stride 3
            def comp(i):
                return unit[:].rearrange("p (a c) -> p a c", c=3)[:, :, i]

            x_, y_, z_ = comp(0), comp(1), comp(2)

            # squares
            x2 = cpool.tile([NP, N], dt.float32)
            nc.vector.tensor_mul(x2[:], x_, x_)
            y2 = cpool.tile([NP, N], dt.float32)
            nc.vector.tensor_mul(y2[:], y_, y_)
            z2 = cpool.tile([NP, N], dt.float32)
            nc.vector.tensor_mul(z2[:], z_, z_)
            r2u = cpool.tile([NP, N], dt.float32)
            nc.vector.tensor_add(r2u[:], x2[:], y2[:])
            nc.vector.tensor_add(r2u[:], r2u[:], z2[:])

            # Y[(b,j), (f, a)] : 16 components
            Yt = cpool.tile([NP, NF * N], dt.float32)

            def yslice(fi):
                return Yt[:, fi * N:(fi + 1) * N]

            nc.vector.memset(yslice(0), 0.28209479177387814)
            nc.scalar.mul(yslice(1), y_, 0.4886025119029199)
            nc.scalar.mul(yslice(2), z_, 0.4886025119029199)
            nc.scalar.mul(yslice(3), x_, 0.4886025119029199)
            nc.vector.scalar_tensor_tensor(yslice(4), x_, 1.0925484305920792, y_,
                                           op0=ALU.mult, op1=ALU.mult)
            nc.vector.scalar_tensor_tensor(yslice(5), y_, 1.0925484305920792, z_,
                                           op0=ALU.mult, op1=ALU.mult)
            # Y6 = 0.3154*(3 z2 - r2u) ; Y8 = 0.5463*(x2-y2)
            t6 = cpool.tile([NP, N], dt.float32)
            nc.vector.scalar_tensor_tensor(t6[:], z2[:], 3.0, r2u[:],
                                           op0=ALU.mult, op1=ALU.subtract)
            nc.scalar.mul(yslice(6), t6[:], 0.31539156525252005)
            nc.vector.scalar_tensor_tensor(yslice(7), x_, 1.0925484305920792, z_,
                                           op0=ALU.mult, op1=ALU.mult)
            t8 = cpool.tile([NP, N], dt.float32)
            nc.vector.tensor_sub(t8[:], x2[:], y2[:])
            nc.scalar.mul(yslice(8), t8[:], 0.5462742152960396)
            # l=3
            t9 = cpool.tile([NP, N], dt.float32)   # 3x2 - y2
            nc.vector.scalar_tensor_tensor(t9[:], x2[:], 3.0, y2[:],
                                           op0=ALU.mult, op1=ALU.subtract)
            nc.vector.scalar_tensor_tensor(yslice(9), t9[:], 0.5900435899266435, y_,
                                           op0=ALU.mult, op1=ALU.mult)
            nc.vector.scalar_tensor_tensor(yslice(10), yslice(4),
                                           2.890611442640554 / 1.0925484305920792, z_,
                                           op0=ALU.mult, op1=ALU.mult)
            t11 = cpool.tile([NP, N], dt.float32)  # 5 z2 - r2u
            nc.vector.scalar_tensor_tensor(t11[:], z2[:], 5.0, r2u[:],
                                           op0=ALU.mult, op1=ALU.subtract)
            nc.vector.scalar_tensor_tensor(yslice(11), t11[:], 0.4570457994644658, y_,
                                           op0=ALU.mult, op1=ALU.mult)
            t12 = cpool.tile([NP, N], dt.float32)  # (5/3) z2 - r2u
            nc.vector.scalar_tensor_tensor(t12[:], z2[:], 5.0 / 3.0, r2u[:],
                                           op0=ALU.mult, op1=ALU.subtract)
            nc.vector.scalar_tensor_tensor(yslice(12), t12[:],
                                           3.0 * 0.3731763325901154, z_,
                                           op0=ALU.mult, op1=ALU.mult)
            nc.vector.scalar_tensor_tensor(yslice(13), t11[:], 0.4570457994644658, x_,
                                           op0=ALU.mult, op1=ALU.mult)
            nc.vector.scalar_tensor_tensor(yslice(14), t8[:], 1.445305721320277, z_,
                                           op0=ALU.mult, op1=ALU.mult)
            t15 = cpool.tile([NP, N], dt.float32)  # x2 - 3 y2
            nc.vector.scalar_tensor_tensor(t15[:], y2[:], -3.0, x2[:],
                                           op0=ALU.mult, op1=ALU.add)
            nc.vector.scalar_tensor_tensor(yslice(15), t15[:], 0.5900435899266435, x_,
                                           op0=ALU.mult, op1=ALU.mult)

            # radial basis * mask_j : rbm[(b,j), (k,a)]
            dgrid = cpool.tile([NP, NB * N], dt.float32)
            nc.vector.tensor_tensor(
                dgrid[:].rearrange("p (k a) -> p k a", a=N),
                radii[:].unsqueeze(1).broadcast_to([NP, NB, N]),
                cgrid[:].rearrange("p (k a) -> p k a", a=N),
                op=ALU.subtract)
            nc.vector.tensor_mul(dgrid[:], dgrid[:], dgrid[:])
            rbm = cpool.tile([NP, NB * N], dt.float32)
            nc.scalar.activation(rbm[:], dgrid[:], ACT.Exp,
                                 scale=-0.5 / (WIDTH * WIDTH))
            nc.vector.tensor_scalar_mul(rbm[:], rbm[:], maskp[:, 0:1])

            # Filter[(b,j), (k, f, a)] = rbm (bc f) * Y (bc k)
            filt = cpool.tile([NP, NB * NF * N], s1_dt)
            nc.vector.tensor_tensor(
                filt[:].rearrange("p (k f a) -> p k f a", k=NB, f=NF),
                rbm[:].rearrange("p (k a) -> p k a", a=N)
                    .unsqueeze(2).broadcast_to([NP, NB, NF, N]),
                Yt[:].rearrange("p (f a) -> p f a", a=N)
                    .unsqueeze(1).broadcast_to([NP, NB, NF, N]),
                op=ALU.mult)

            # ================= embedding (layer-0 features) =================
            onehotT = cpool.tile([MAXZ, NP], dt.float32)
            nc.vector.tensor_scalar(onehotT[:], zrep[:], iota10[:, 0:1], None,
                                    op0=ALU.is_equal)
            ps_emb = psB.tile([NP, EMBED], dt.float32, bufs=1)
            nc.tensor.matmul(ps_emb[:], onehotT[:], embed[:], start=True, stop=True)
            fin0 = cpool.tile([NP, EMBED], s1_dt)
            nc.vector.tensor_copy(fin0[:], ps_emb[:])
            tap("fin0", fin0[:])
            tap("Yt", Yt[:])
            tap("rbm", rbm[:])
            tap("filt", filt[:])
            tap("unit", unit[:])

            # ================= WQ builds =================
            # Pre-expansions keep every product op at <=3 free dims:
            #   WU[p, (k, uo152)]   = wexp broadcast over o   (per layer)
            #   QU[p, (lf,lo)-blocks of (m, uo)] = qexp broadcast over u (const)
            # WQ block = WU (bc over m) * QU (bc over k).
            qu_h = cpool.tile([128, 16 * 152], wq_in_dt)   # (lf-major, 152 per lf? no: per (lf,lo) block)
            # layout: for (lf,lo): block at QUOFF[(lf,lo)] of size nmf*u*no
            quoff = {}
            _qo = 0
            for (lf, lo) in LFLO_H:
                nmf, no, u = 2 * lf + 1, 2 * lo + 1, U[lo]
                quoff[(lf, lo)] = _qo
                out_ap = (qu_h[:, _qo:_qo + nmf * u * no]
                          .rearrange("p (m u o) -> p m u o", m=nmf, u=u))
                in_ap = (qh[:, QOFF_H[(lf, lo)]:QOFF_H[(lf, lo)] + nmf * no]
                         .rearrange("p (m o) -> p m o", o=no)
                         .unsqueeze(2).broadcast_to([128, nmf, u, no]))
                nc.vector.tensor_copy(out_ap, in_ap)
                _qo += nmf * u * no
            qu0_cols = sum((2 * lf + 1) * U[lo] * (2 * lo + 1) for lf, lo in LFLO_0)
            qu0 = cpool.tile([32, qu0_cols], wq_in_dt)
            qu0off = {}
            _qo = 0
            for (lf, lo) in LFLO_0:
                nmf, no, u = 2 * lf + 1, 2 * lo + 1, U[lo]
                qu0off[(lf, lo)] = _qo
                out_ap = (qu0[:, _qo:_qo + nmf * u * no]
                          .rearrange("p (m u o) -> p m u o", m=nmf, u=u))
                in_ap = (q0[:, QOFF_0[(lf, lo)]:QOFF_0[(lf, lo)] + nmf * no]
                         .rearrange("p (m o) -> p m o", o=no)
                         .unsqueeze(2).broadcast_to([32, nmf, u, no]))
                nc.vector.tensor_copy(out_ap, in_ap)
                _qo += nmf * u * no

            # WU has a distinct (k, u*no) block per (lf, lo) — radial weights
            # are per-path, so the o-broadcast expansion cannot be shared
            # across lf.
            wuoff = {}
            _wo = 0
            for (lf, lo) in LFLO_H:
                wuoff[(lf, lo)] = _wo
                _wo += NB * U[lo] * (2 * lo + 1)
            WU_COLS = _wo
            wu0off = {}
            _wo = 0
            for (lf, lo) in LFLO_0:
                wu0off[(lf, lo)] = _wo
                _wo += NB * U[lo] * (2 * lo + 1)
            WU0_COLS = _wo

            def make_wu(L):
                nvi = 32 if L == 0 else 128
                woff = WOFF_0 if L == 0 else WOFF_H
                uoff = wu0off if L == 0 else wuoff
                lflo = LFLO_0 if L == 0 else LFLO_H
                ncols = WU0_COLS if L == 0 else WU_COLS
                t = cpool.tile([nvi, ncols], wq_in_dt, name=f"wu{L}")
                for wi, (lf, lo) in enumerate(lflo):
                    no, u = 2 * lo + 1, U[lo]
                    out_ap = (t[:, uoff[(lf, lo)]:uoff[(lf, lo)] + NB * u * no]
                              .rearrange("p (k u o) -> p k u o", k=NB, u=u))
                    in_ap = (wexp[L][:, woff[(lf, lo)]:woff[(lf, lo)] + NB * u]
                             .rearrange("p (k u) -> p k u", u=u)
                             .unsqueeze(3).broadcast_to([nvi, NB, u, no]))
                    eng = nc.vector if wi % 2 == 0 else nc.gpsimd
                    eng.tensor_copy(out_ap, in_ap)
                return t

            wu = [make_wu(L) for L in range(3)]

            tap("qu_h", qu_h[:])
            tap("wu1", wu[1][:])
            wq0 = wqpool.tile([32, WQ0_COLS], wq_dt)
            for (lf, lo) in LFLO_0:
                nmf, no, u = 2 * lf + 1, 2 * lo + 1, U[lo]
                uo = u * no
                w_ap = (wu[0][:, wu0off[(lf, lo)]:wu0off[(lf, lo)] + NB * uo]
                        .rearrange("p (k c) -> p k c", k=NB)
                        .unsqueeze(2).broadcast_to([32, NB, nmf, uo]))
                q_ap = (qu0[:, qu0off[(lf, lo)]:qu0off[(lf, lo)] + nmf * uo]
                        .rearrange("p (m c) -> p m c", m=nmf)
                        .unsqueeze(1).broadcast_to([32, NB, nmf, uo]))
                out_ap = (wq0[:, W0QOFF[(lf, lo)]:W0QOFF[(lf, lo)] + NB * nmf * uo]
                          .rearrange("p (k m c) -> p k m c", k=NB, m=nmf))
                nc.vector.tensor_tensor(out_ap, w_ap, q_ap, op=ALU.mult)

            wqh = []
            for L in (1, 2):
                t = wqpool.tile([128, WQ_H_COLS], wq_dt, name=f"wqh{L}",
                                tag="wqh", bufs=1)
                wqh.append(t)
                for bi, (lf, lo) in enumerate(LFLO_H):
                    nmf, no, u = 2 * lf + 1, 2 * lo + 1, U[lo]
                    uo = u * no
                    eng = nc.vector if bi % 2 == 0 else nc.gpsimd
                    w_ap = (wu[L][:, wuoff[(lf, lo)]:wuoff[(lf, lo)] + NB * uo]
                            .rearrange("p (k c) -> p k c", k=NB)
                            .unsqueeze(2).broadcast_to([128, NB, nmf, uo]))
                    q_ap = (qu_h[:, quoff[(lf, lo)]:quoff[(lf, lo)] + nmf * uo]
                            .rearrange("p (m c) -> p m c", m=nmf)
                            .unsqueeze(1).broadcast_to([128, NB, nmf, uo]))
                    out_ap = (t[:].rearrange("p (k f c) -> p k f c", k=NB, f=NF)
                              [:, :, F_OFF[lf]:F_OFF[lf] + nmf,
                               UO_OFF[lo]:UO_OFF[lo] + uo])
                    eng.tensor_tensor(out_ap, w_ap, q_ap, op=ALU.mult)

            tap("wq0", wq0[:])
            tap("wqh1", wqh[0][:])

            # ================= layers =================
            G = gpool.tile([128, NB * NF * 128], wq_dt)   # [vi, (k,f, a128)]
            NCHUNK = 8
            CC = NB * NF * N // NCHUNK                    # 480 filter cols / chunk
            KFC = CC // N                                  # 20 (k,f) groups / chunk

            fin = fin0
            for L in range(3):
                nvi = EMBED if L == 0 else 128
                # ---- stage 1: G = fin^T @ Filter, per molecule row-group ----
                for ch in range(NCHUNK):
                    for b in range(BC):
                        psg = psA.tile([128, CC], dt.float32, name="psg", tag="psg")
                        lhsT = fin[:].rearrange("(b j) v -> b j v", j=PAD)[b, 0:N, :]
                        rhs = (filt[:].rearrange("(b j) c -> b j c", j=PAD)
                               [b, 0:N, ch * CC:(ch + 1) * CC])
                        nc.tensor.matmul(psg[0:nvi, :], lhsT, rhs,
                                         start=True, stop=True,
                                         tile_position=(PAD * b, 0))
                        # evacuate into G (bf16) at cols kf*128 + 32*b + a
                        out_ap = (G[:].rearrange("p (g c) -> p g c", c=128)
                                  [0:nvi, ch * KFC:(ch + 1) * KFC,
                                   PAD * b:PAD * b + N])
                        in_ap = psg[0:nvi, :].rearrange("p (g a) -> p g a", a=N)
                        if (ch * BC + b) % 2 == 0:
                            nc.vector.tensor_copy(out_ap, in_ap)
                        else:
                            nc.scalar.copy(out_ap, in_ap)

                tap(f"G{L}", G[:])
                # ---- stage 2: accumulate over (k,f) ----
                pso = psB.tile([128, 152], dt.float32, name="pso", tag="pso")
                if L == 0:
                    for (lf, lo) in LFLO_0:
                        nmf, no, u = 2 * lf + 1, 2 * lo + 1, U[lo]
                        steps = [(k, mf) for k in range(NB) for mf in range(nmf)]
                        for si, (k, mf) in enumerate(steps):
                            kf = k * NF + F_OFF[lf] + mf
                            rhs = (wq0[:, W0QOFF[(lf, lo)] + (si * u * no):
                                       W0QOFF[(lf, lo)] + (si + 1) * u * no])
                            nc.tensor.matmul(
                                pso[:, UO_OFF[lo]:UO_OFF[lo] + u * no],
                                G[0:nvi, kf * 128:(kf + 1) * 128],
                                rhs,
                                start=(si == 0), stop=(si == len(steps) - 1))
                else:
                    wq = wqh[L - 1]
                    for kf in range(NB * NF):
                        nc.tensor.matmul(
                            pso[:],
                            G[:, kf * 128:(kf + 1) * 128],
                            wq[:, kf * 152:(kf + 1) * 152],
                            start=(kf == 0), stop=(kf == NB * NF - 1))

                tap(f"pso{L}", pso[:])
                # ---- gating ----
                fnew = cpool.tile([NP, 128], s1_dt, name=f"fin{L + 1}")
                gates = cpool.tile([NP, 24], dt.float32, name=f"gates{L}")
                nc.scalar.activation(gates[:], pso[:, 24:48], ACT.Sigmoid)
                nc.vector.tensor_scalar_mul(gates[:], gates[:], maskp[:, 0:1])
                nc.vector.scalar_tensor_tensor(
                    fnew[:, 0:24], pso[:, 0:24], 0.0,
                    maskp[:, 0:1].broadcast_to([NP, 24]),
                    op0=ALU.max, op1=ALU.mult)
                goff = 0
                for l in range(1, 4):
                    m, no = MULS[l], 2 * l + 1
                    nc.vector.tensor_tensor(
                        fnew[:, VI_OFF[l]:VI_OFF[l + 1]]
                            .rearrange("p (v o) -> p v o", o=no),
                        pso[:, UO_OFF[l]:UO_OFF[l + 1]]
                            .rearrange("p (v o) -> p v o", o=no),
                        gates[:, goff:goff + m].unsqueeze(2)
                            .broadcast_to([NP, m, no]),
                        op=ALU.mult)
                    goff += m
                fin = fnew
                tap(f"fin{L + 1}", fnew[:])

            # ---- store output ----
            for b in range(BC):
                nc.sync.dma_start(
                    d_out.ap()[b * N:(b + 1) * N, :],
                    fin[:].bitcast(dt.float32)
                        .rearrange("(b j) v -> b j v", j=PAD)[b, 0:N, :])

    nc.compile()
    return nc


_NC = None
_NC_TAPS = None


def _get_nc(debug_taps=()):
    global _NC, _NC_TAPS
    key = tuple(sorted(debug_taps))
    if _NC is None or _NC_TAPS != key:
        _NC = _build_nc(debug_taps=key)
        _NC_TAPS = key
    return _NC


# ---------------------------------------------------------------------------
# entry point
# ---------------------------------------------------------------------------
def kernel(z, pos, mask, embed_table, radial_w, _trace=False, _trace_kwargs=None,
           _debug_taps=()):
    from concourse.bass_utils import run_bass_kernel_spmd

    z = np.asarray(z)
    pos = np.asarray(pos, np.float32)
    mask = np.asarray(mask, np.float32)
    embed_table = np.asarray(embed_table, np.float32)
    radial_w = np.asarray(radial_w, np.float32)

    wq_np = np.float32
    if WQ_BF16_IN:
        import ml_dtypes
        wq_np = ml_dtypes.bfloat16

    wexp = [_build_wexp(radial_w, L).astype(wq_np) for L in range(3)]
    q0 = _Q0_CONST.astype(wq_np)
    qh = _QH_CONST.astype(wq_np)

    in_maps = []
    for c in range(NCORES):
        zc = np.asarray(z[c * BC:(c + 1) * BC], np.float32)      # [4,24]
        pc = pos[c * BC:(c + 1) * BC]                            # [4,24,3]
        mc = mask[c * BC:(c + 1) * BC]                           # [4,24]
        zp = np.zeros((BC, PAD), np.float32)
        zp[:, :N] = zc
        pj = np.zeros((BC, PAD, 3), np.float32)
        pj[:, :N, :] = pc
        pa = np.zeros((BC, PAD, N * 3), np.float32)
        pa[:, :, :] = pc.reshape(BC, 1, N * 3)
        mp = np.zeros((BC, PAD), np.float32)
        mp[:, :N] = mc
        in_maps.append({
            "zrep": np.tile(zp.reshape(1, NP), (MAXZ, 1)),
            "posj": pj.reshape(NP, 3),
            "posa": pa.reshape(NP, N * 3),
            "maskp": mp.reshape(NP, 1),
            "embed": embed_table,
            "cgrid": _CGRID_CONST,
            "iota10": _IOTA10_CONST,
            "wexp0": wexp[0], "wexp1": wexp[1], "wexp2": wexp[2],
            "qexp0": q0, "qexph": qh,
            "cbias": np.tile(np.array([[0.0, 1e-12]], np.float32), (128, 1)),
        })

    nc = _get_nc(_debug_taps)
    kw = {}
    if _trace:
        kw = dict(trace=True, **(_trace_kwargs or {}))
    res = run_bass_kernel_spmd(nc, in_maps, core_ids=list(range(NCORES)), **kw)
    out = np.concatenate(
        [res.results[c]["out"].reshape(BC, N, 128) for c in range(NCORES)], axis=0)
    if _debug_taps:
        taps = {k[4:]: [res.results[c][k] for c in range(NCORES)]
                for k in res.results[0] if k.startswith("tap_")}
        return out.astype(np.float32), taps
    if _trace:
        return out.astype(np.float32), res
    return out.astype(np.float32)
